# revision 21
# baseline (speedup 1.0000x reference)
"""Trainium2 Bass kernel for nn_MetaComprehensiveRegularization.

reference math (fp32):
  loss_common  = -sum(zc * zc)
  loss_special = -sum_v sum_i dot(zc_i, zs_vi) / (||zc_i|| * ||zs_vi||)
  output = stack([loss_common, loss_special])  # shape [2]

Strategy: data-parallel shard along N across 8 cores. Each core computes
row-wise reductions only — ||zc_row||^2 and dot(zc,zs) on DVE
(scalar_tensor_tensor + accum), ||zs_row||^2 on ACT (Square + accum_out) —
and ships the raw per-row stats to the host, which combines them into the
two scalars in float64. Raw Bacc program (no TileContext) with hand-rolled
semaphores: per-input-tile DMA sems, per-engine tile-release counters for
SBUF slot reuse, and trailing nops sequencing the output DMAs.
"""

from contextlib import ExitStack

import numpy as np

N_CORES = 8
N, D, V = 16384, 512, 4
N_LOC = N // N_CORES      # 2048 rows per core
P = 128                   # SBUF partitions
BLOCKS = N_LOC // P       # 16 row-blocks per core
A = 4                     # row-blocks per DMA chunk (1 MiB per dma_start)
CHUNKS = BLOCKS // A      # 4
NB = V * BLOCKS           # 64 zs-stat columns
ZS_SLOTS = 8              # zs SBUF ring depth (tiles of [128, A, D])
N_DMA = CHUNKS * (1 + V)  # 20 input DMAs

_PROGRAM = None


def _build_program():
    import concourse.bacc as bacc
    from concourse import mybir

    f32 = mybir.dt.float32
    nc = bacc.Bacc(
        "TRN2", target_bir_lowering=False, debug=False, num_devices=N_CORES
    )
    zc_t = nc.dram_tensor("zc", [N_LOC, D], f32, kind="ExternalInput")
    zs_t = nc.dram_tensor("zs", [V, N_LOC, D], f32, kind="ExternalInput")
    outv_t = nc.dram_tensor("outv", [P, BLOCKS + NB], f32, kind="ExternalOutput")
    outs_t = nc.dram_tensor("outs", [P, NB], f32, kind="ExternalOutput")

    # row n = c*(A*P) + a*P + p  ->  view [c, p, a, d]
    zc_v = zc_t.ap().rearrange("(c a p) d -> c p a d", a=A, p=P)
    zs_v = zs_t.ap().rearrange("v (c a p) d -> v c p a d", a=A, p=P)

    mult = mybir.AluOpType.mult

    with ExitStack() as ctx:
        zc_slots = [
            ctx.enter_context(nc.sbuf_tensor(f"zc{i}", [P, A, D], f32))
            for i in range(2)
        ]
        zs_slots = [
            ctx.enter_context(nc.sbuf_tensor(f"zs{i}", [P, A, D], f32))
            for i in range(ZS_SLOTS)
        ]
        stats_v = ctx.enter_context(nc.sbuf_tensor("sv", [P, BLOCKS + NB], f32))
        stats_s = ctx.enter_context(nc.sbuf_tensor("ss", [P, NB], f32))
        # One dead-write sink per op: the race detector treats same-engine
        # WAW between pipelined ops as a hazard, so don't share sinks.
        dummy_v = ctx.enter_context(
            nc.sbuf_tensor("dv", [P, BLOCKS + NB], f32)
        )
        dummy_s = ctx.enter_context(nc.sbuf_tensor("ds", [P, NB], f32))

        dma_sems = [
            ctx.enter_context(nc.semaphore(f"dma{i}")) for i in range(N_DMA)
        ]
        sem_vz = ctx.enter_context(nc.semaphore("vz"))  # DVE op completions
        sem_sz = ctx.enter_context(nc.semaphore("sz"))  # ACT op completions
        sem_out = ctx.enter_context(nc.semaphore("out"))

        # vector op index after which zs tile k is fully DVE-consumed:
        # per chunk-set c: 4 zc-squares then 4 dots per v.
        def vz_done(k):
            c, v = k // V, k % V
            return 20 * c + 4 * (v + 1) + 4

        # scalar op index after which zs tile k is fully ACT-consumed
        def sz_done(k):
            return 4 * k + 4

        # ---- sync engine: all input DMAs, flow-controlled by slot reuse ----
        # Ramp: concurrently-issued DMAs share HBM bandwidth, which delays the
        # first tiles that gate compute start. Serialize the first zc/zs tile
        # as interleaved per-block sub-DMAs so each lands at full bandwidth
        # and both engines start ~6us earlier. Strict serialization also makes
        # the cumulative sub-thresholds on one sem race-free.
        for a in range(A):
            nc.sync.dma_start(
                out=zc_slots[0].ap()[:, a, :], in_=zc_v[0, :, a, :]
            ).then_inc(dma_sems[0], 16)
            nc.sync.wait_ge(dma_sems[0], 16 * (a + 1))
            nc.sync.dma_start(
                out=zs_slots[0].ap()[:, a, :], in_=zs_v[0, 0, :, a, :]
            ).then_inc(dma_sems[1], 16)
            nc.sync.wait_ge(dma_sems[1], 16 * (a + 1))
        for c in range(CHUNKS):
            if c >= 2:
                # zc slot c%2 was last read by the final dot of chunk-set c-2
                nc.sync.wait_ge(sem_vz, 20 * (c - 1))
            if c > 0:
                nc.sync.dma_start(
                    out=zc_slots[c % 2].ap(), in_=zc_v[c]
                ).then_inc(dma_sems[5 * c], 16)
            for v in range(V):
                k = 4 * c + v
                if k == 0:
                    continue
                if k >= ZS_SLOTS:
                    nc.sync.wait_ge(sem_vz, vz_done(k - ZS_SLOTS))
                    nc.sync.wait_ge(sem_sz, sz_done(k - ZS_SLOTS))
                nc.sync.dma_start(
                    out=zs_slots[k % ZS_SLOTS].ap(), in_=zs_v[v, c]
                ).then_inc(dma_sems[5 * c + 1 + v], 16)

        # ---- vector engine: zc squares + dots ----
        for c in range(CHUNKS):
            zc_tile = zc_slots[c % 2].ap()
            if c > 0:
                nc.vector.wait_ge(dma_sems[5 * c], 16)
            for a in range(A):
                if c == 0:
                    nc.vector.wait_ge(dma_sems[0], 16 * (a + 1))
                t = c * A + a
                nc.vector.scalar_tensor_tensor(
                    out=dummy_v.ap()[:, t : t + 1].broadcast_to((P, D)),
                    in0=zc_tile[:, a, :],
                    scalar=1.0,
                    in1=zc_tile[:, a, :],
                    op0=mult,
                    op1=mult,
                    accum_out=stats_v.ap()[:, t : t + 1],
                ).then_inc(sem_vz, 1)
            for v in range(V):
                k = 4 * c + v
                zs_tile = zs_slots[k % ZS_SLOTS].ap()
                if k > 0:
                    nc.vector.wait_ge(dma_sems[5 * c + 1 + v], 16)
                for a in range(A):
                    if k == 0:
                        nc.vector.wait_ge(dma_sems[1], 16 * (a + 1))
                    col = BLOCKS + v * BLOCKS + c * A + a
                    nc.vector.scalar_tensor_tensor(
                        out=dummy_v.ap()[:, col : col + 1].broadcast_to((P, D)),
                        in0=zc_tile[:, a, :],
                        scalar=1.0,
                        in1=zs_tile[:, a, :],
                        op0=mult,
                        op1=mult,
                        accum_out=stats_v.ap()[:, col : col + 1],
                    ).then_inc(sem_vz, 1)

        # ---- scalar engine: zs squares ----
        for c in range(CHUNKS):
            for v in range(V):
                k = 4 * c + v
                zs_tile = zs_slots[k % ZS_SLOTS].ap()
                if k > 0:
                    nc.scalar.wait_ge(dma_sems[5 * c + 1 + v], 16)
                for a in range(A):
                    if k == 0:
                        nc.scalar.wait_ge(dma_sems[1], 16 * (a + 1))
                    col = v * BLOCKS + c * A + a
                    nc.scalar.activation(
                        out=dummy_s.ap()[:, col : col + 1].broadcast_to((P, D)),
                        in_=zs_tile[:, a, :],
                        func=mybir.ActivationFunctionType.Square,
                        accum_out=stats_s.ap()[:, col : col + 1],
                    ).then_inc(sem_sz, 1)

        # ---- output DMAs after both engines fully done ----
        nc.sync.wait_ge(sem_vz, 20 * CHUNKS)
        nc.sync.wait_ge(sem_sz, 16 * CHUNKS)
        nc.sync.dma_start(out=outv_t.ap(), in_=stats_v.ap()).then_inc(sem_out, 16)
        nc.sync.dma_start(out=outs_t.ap(), in_=stats_s.ap()).then_inc(sem_out, 16)
        nc.sync.wait_ge(sem_out, 32)

    nc.compile()
    return nc


def _get_program():
    global _PROGRAM
    if _PROGRAM is None:
        _PROGRAM = _build_program()
    return _PROGRAM


def _combine(stats_v: np.ndarray, stats_s: np.ndarray) -> tuple[float, float]:
    """stats_v: [cores, P, 16+64] ([cn2|dot]), stats_s: [cores, P, 64] (sn2)."""
    sv = stats_v.astype(np.float64)
    cn2 = sv[:, :, :BLOCKS]                         # [cores, P, 16]
    dot = sv[:, :, BLOCKS:]                         # [cores, P, 64]
    sn2 = stats_s.astype(np.float64)                # [cores, P, 64]
    common = cn2.sum()
    eps = 1e-12
    cn = np.maximum(np.sqrt(cn2), eps)              # [cores, P, 16]
    sn = np.maximum(np.sqrt(sn2), eps)              # [cores, P, 64]
    v_cn = np.tile(cn, (1, 1, V))                   # align with v*16+t layout
    special = (dot / (v_cn * sn)).sum()
    return common, special


def kernel(zc: np.ndarray, zs: np.ndarray) -> np.ndarray:
    from concourse.bass_utils import run_bass_kernel_spmd

    zc = np.ascontiguousarray(np.asarray(zc), dtype=np.float32)
    zs = np.ascontiguousarray(np.asarray(zs), dtype=np.float32)
    assert zc.shape == (N, D) and zs.shape == (V, N, D)

    nc = _get_program()
    in_maps = [
        {
            "zc": np.ascontiguousarray(zc[i * N_LOC : (i + 1) * N_LOC]),
            "zs": np.ascontiguousarray(zs[:, i * N_LOC : (i + 1) * N_LOC]),
        }
        for i in range(N_CORES)
    ]
    res = run_bass_kernel_spmd(nc, in_maps, core_ids=list(range(N_CORES)))
    stats_v = np.stack([r["outv"] for r in res.results])  # [8, 128, 80]
    stats_s = np.stack([r["outs"] for r in res.results])  # [8, 128, 64]
    common, special = _combine(stats_v, stats_s)
    return np.asarray([-common, -special], dtype=np.float32)


# revision 26
# speedup vs baseline: 1.3367x; 1.3367x over previous
"""Trainium2 Bass kernel for nn_MetaComprehensiveRegularization.

reference math (fp32):
  loss_common  = -sum(zc * zc)
  loss_special = -sum_v sum_i dot(zc_i, zs_vi) / (||zc_i|| * ||zs_vi||)
  output = stack([loss_common, loss_special])  # shape [2]

Strategy: data-parallel shard along N across 8 cores. Each core computes
row-wise reductions only — ||zc_row||^2 and dot(zc,zs) on DVE
(scalar_tensor_tensor + accum), ||zs_row||^2 on ACT (Square + accum_out) —
and ships the raw per-row stats to the host, which combines them into the
two scalars in float64. Raw Bacc program (no TileContext) with hand-rolled
semaphores: per-input-tile DMA sems, per-engine tile-release counters for
SBUF slot reuse, and trailing nops sequencing the output DMAs.
"""

from contextlib import ExitStack

import numpy as np

N_CORES = 8
N, D, V = 16384, 512, 4
N_LOC = N // N_CORES      # 2048 rows per core
P = 128                   # SBUF partitions
BLOCKS = N_LOC // P       # 16 row-blocks per core
A = 4                     # row-blocks per DMA chunk (1 MiB per dma_start)
CHUNKS = BLOCKS // A      # 4
NB = V * BLOCKS           # 64 zs-stat columns
ZS_SLOTS = 8              # zs SBUF ring depth (tiles of [128, A, D])
N_DMA = CHUNKS * (1 + V)  # 20 input DMAs

_PROGRAM = None


def _build_program():
    import concourse.bacc as bacc
    from concourse import mybir

    f32 = mybir.dt.float32
    nc = bacc.Bacc(
        "TRN2", target_bir_lowering=False, debug=False, num_devices=N_CORES
    )
    zc_t = nc.dram_tensor("zc", [N_LOC, D], f32, kind="ExternalInput")
    zs_t = nc.dram_tensor("zs", [V, N_LOC, D], f32, kind="ExternalInput")
    outv_t = nc.dram_tensor("outv", [P, BLOCKS + NB], f32, kind="ExternalOutput")
    outs_t = nc.dram_tensor("outs", [P, NB], f32, kind="ExternalOutput")

    # row n = c*(A*P) + a*P + p  ->  view [c, p, a, d]
    zc_v = zc_t.ap().rearrange("(c a p) d -> c p a d", a=A, p=P)
    zs_v = zs_t.ap().rearrange("v (c a p) d -> v c p a d", a=A, p=P)

    mult = mybir.AluOpType.mult

    with ExitStack() as ctx:
        zc_slots = [
            ctx.enter_context(nc.sbuf_tensor(f"zc{i}", [P, A, D], f32))
            for i in range(2)
        ]
        zs_slots = [
            ctx.enter_context(nc.sbuf_tensor(f"zs{i}", [P, A, D], f32))
            for i in range(ZS_SLOTS)
        ]
        stats_v = ctx.enter_context(nc.sbuf_tensor("sv", [P, BLOCKS + NB], f32))
        stats_s = ctx.enter_context(nc.sbuf_tensor("ss", [P, NB], f32))
        # One dead-write sink per op: the race detector treats same-engine
        # WAW between pipelined ops as a hazard, so don't share sinks.
        dummy_v = ctx.enter_context(
            nc.sbuf_tensor("dv", [P, BLOCKS + NB], f32)
        )
        dummy_s = ctx.enter_context(nc.sbuf_tensor("ds", [P, NB], f32))

        dma_sems = [
            ctx.enter_context(nc.semaphore(f"dma{i}")) for i in range(N_DMA)
        ]
        # Per-sub-block sems for the first zc/zs tile (completion order of
        # concurrent transfers is not guaranteed, so one counting sem per
        # sub-DMA).
        zc0_sems = [ctx.enter_context(nc.semaphore(f"zc0a{a}")) for a in range(A)]
        zs0_sems = [ctx.enter_context(nc.semaphore(f"zs0a{a}")) for a in range(A)]
        sem_vz = ctx.enter_context(nc.semaphore("vz"))  # DVE op completions
        sem_sz = ctx.enter_context(nc.semaphore("sz"))  # ACT op completions
        sem_out = ctx.enter_context(nc.semaphore("out"))

        # vector op index after which zs tile k is fully DVE-consumed:
        # per chunk-set c: 4 zc-squares then 4 dots per v.
        def vz_done(k):
            c, v = k // V, k % V
            return 20 * c + 4 * (v + 1) + 4

        # scalar op index after which zs tile k is fully ACT-consumed
        def sz_done(k):
            return 4 * k + 4

        # ---- sync engine: all input DMAs, flow-controlled by slot reuse ----
        # Ramp: concurrently-issued DMAs share HBM bandwidth, which delays the
        # first tiles that gate compute start. Serialize the first zc/zs tile
        # as interleaved per-block sub-DMAs so each lands at full bandwidth
        # and both engines start ~6us earlier. Strict serialization also makes
        # the cumulative sub-thresholds on one sem race-free.
        for a in range(A):
            nc.sync.dma_start(
                out=zc_slots[0].ap()[:, a, :], in_=zc_v[0, :, a, :]
            ).then_inc(zc0_sems[a], 16)
            nc.sync.dma_start(
                out=zs_slots[0].ap()[:, a, :], in_=zs_v[0, 0, :, a, :]
            ).then_inc(zs0_sems[a], 16)
        for c in range(CHUNKS):
            if c >= 2:
                # zc slot c%2 was last read by the final dot of chunk-set c-2
                nc.sync.wait_ge(sem_vz, 20 * (c - 1))
            if c > 0:
                nc.sync.dma_start(
                    out=zc_slots[c % 2].ap(), in_=zc_v[c]
                ).then_inc(dma_sems[5 * c], 16)
            for v in range(V):
                k = 4 * c + v
                if k == 0:
                    continue
                if k >= ZS_SLOTS:
                    nc.sync.wait_ge(sem_vz, vz_done(k - ZS_SLOTS))
                    nc.sync.wait_ge(sem_sz, sz_done(k - ZS_SLOTS))
                nc.sync.dma_start(
                    out=zs_slots[k % ZS_SLOTS].ap(), in_=zs_v[v, c]
                ).then_inc(dma_sems[5 * c + 1 + v], 16)

        # ---- vector engine: zc squares + dots ----
        for c in range(CHUNKS):
            zc_tile = zc_slots[c % 2].ap()
            if c > 0:
                nc.vector.wait_ge(dma_sems[5 * c], 16)
            for a in range(A):
                if c == 0:
                    nc.vector.wait_ge(zc0_sems[a], 16)
                t = c * A + a
                nc.vector.scalar_tensor_tensor(
                    out=dummy_v.ap()[:, t : t + 1].broadcast_to((P, D)),
                    in0=zc_tile[:, a, :],
                    scalar=1.0,
                    in1=zc_tile[:, a, :],
                    op0=mult,
                    op1=mult,
                    accum_out=stats_v.ap()[:, t : t + 1],
                ).then_inc(sem_vz, 1)
            for v in range(V):
                k = 4 * c + v
                zs_tile = zs_slots[k % ZS_SLOTS].ap()
                if k > 0:
                    nc.vector.wait_ge(dma_sems[5 * c + 1 + v], 16)
                for a in range(A):
                    if k == 0:
                        nc.vector.wait_ge(zs0_sems[a], 16)
                    col = BLOCKS + v * BLOCKS + c * A + a
                    nc.vector.scalar_tensor_tensor(
                        out=dummy_v.ap()[:, col : col + 1].broadcast_to((P, D)),
                        in0=zc_tile[:, a, :],
                        scalar=1.0,
                        in1=zs_tile[:, a, :],
                        op0=mult,
                        op1=mult,
                        accum_out=stats_v.ap()[:, col : col + 1],
                    ).then_inc(sem_vz, 1)

        # ---- scalar engine: zs squares ----
        for c in range(CHUNKS):
            for v in range(V):
                k = 4 * c + v
                zs_tile = zs_slots[k % ZS_SLOTS].ap()
                if k > 0:
                    nc.scalar.wait_ge(dma_sems[5 * c + 1 + v], 16)
                for a in range(A):
                    if k == 0:
                        nc.scalar.wait_ge(zs0_sems[a], 16)
                    col = v * BLOCKS + c * A + a
                    nc.scalar.activation(
                        out=dummy_s.ap()[:, col : col + 1].broadcast_to((P, D)),
                        in_=zs_tile[:, a, :],
                        func=mybir.ActivationFunctionType.Square,
                        accum_out=stats_s.ap()[:, col : col + 1],
                    ).then_inc(sem_sz, 1)

        # ---- output DMAs after both engines fully done ----
        nc.sync.wait_ge(sem_vz, 20 * CHUNKS)
        nc.sync.wait_ge(sem_sz, 16 * CHUNKS)
        nc.sync.dma_start(out=outv_t.ap(), in_=stats_v.ap()).then_inc(sem_out, 16)
        nc.sync.dma_start(out=outs_t.ap(), in_=stats_s.ap()).then_inc(sem_out, 16)
        nc.sync.wait_ge(sem_out, 32)

    nc.compile()
    return nc


def _get_program():
    global _PROGRAM
    if _PROGRAM is None:
        _PROGRAM = _build_program()
    return _PROGRAM


def _combine(stats_v: np.ndarray, stats_s: np.ndarray) -> tuple[float, float]:
    """stats_v: [cores, P, 16+64] ([cn2|dot]), stats_s: [cores, P, 64] (sn2)."""
    sv = stats_v.astype(np.float64)
    cn2 = sv[:, :, :BLOCKS]                         # [cores, P, 16]
    dot = sv[:, :, BLOCKS:]                         # [cores, P, 64]
    sn2 = stats_s.astype(np.float64)                # [cores, P, 64]
    common = cn2.sum()
    eps = 1e-12
    cn = np.maximum(np.sqrt(cn2), eps)              # [cores, P, 16]
    sn = np.maximum(np.sqrt(sn2), eps)              # [cores, P, 64]
    v_cn = np.tile(cn, (1, 1, V))                   # align with v*16+t layout
    special = (dot / (v_cn * sn)).sum()
    return common, special


def kernel(zc: np.ndarray, zs: np.ndarray) -> np.ndarray:
    from concourse.bass_utils import run_bass_kernel_spmd

    zc = np.ascontiguousarray(np.asarray(zc), dtype=np.float32)
    zs = np.ascontiguousarray(np.asarray(zs), dtype=np.float32)
    assert zc.shape == (N, D) and zs.shape == (V, N, D)

    nc = _get_program()
    in_maps = [
        {
            "zc": np.ascontiguousarray(zc[i * N_LOC : (i + 1) * N_LOC]),
            "zs": np.ascontiguousarray(zs[:, i * N_LOC : (i + 1) * N_LOC]),
        }
        for i in range(N_CORES)
    ]
    res = run_bass_kernel_spmd(nc, in_maps, core_ids=list(range(N_CORES)))
    stats_v = np.stack([r["outv"] for r in res.results])  # [8, 128, 80]
    stats_s = np.stack([r["outs"] for r in res.results])  # [8, 128, 64]
    common, special = _combine(stats_v, stats_s)
    return np.asarray([-common, -special], dtype=np.float32)
